# revision 10
# baseline (speedup 1.0000x reference)
"""Phi^4 lattice action on Trainium2 (Bass/Tile), 8-core data parallel.

out[b] = sum_i [ (2 + 0.5*M_SQ)*phi^2 + LAM*phi^4 ]
         - 0.5 * sum_{i,s} phi[b,i]*phi[b,shift[s,i]]

For the canonical 64x64 periodic-lattice shift set {+x,-x,+y,-y} the kinetic
term equals -(S_x + S_y) with S_x = sum_i phi[i]*phi[roll_x(i)] (shift-sum
symmetry over the torus), so the gather reduces to two shifted-view products
that are pure access patterns in SBUF - no data movement.

Per 128-row batch tile:
  ACT: a = phi^2 ; junk = Square(sqrt(LAM)*a) with accum_out -> LAM*sum phi^4
  DVE: 4x scalar_tensor_tensor (x-main, x-wrap, y-main, y-wrap), each fused
       multiply(+negate)+reduce into one accumulator column, then one tiny
       reduce over the 5 partial columns.
"""

import json
import math

import numpy as np

import concourse.bass as bass
import concourse.mybir as mybir
import concourse.tile as tile
from concourse.bass_utils import run_bass_kernel_spmd

def _max_waits(opcode: str) -> int:
    # This walrus build accepts at most ONE sync wait per instruction.
    return 1


def _split_excess_waits(bir_bytes: bytes) -> bytes:
    """The container's walrus codegen rejects any instruction carrying more
    than 2 sync waits ("Too many sync wait commands"), but Tile's tail drain
    and WAR-gated DMA loads can carry 3+. Peel excess waits onto injected
    same-engine Drain instructions placed immediately before the offender."""
    bir = json.loads(bir_bytes)
    n_new = 0
    for func in bir.get("functions", []):
        for bb in func.get("blocks", []):
            insts = bb.get("instructions", [])
            out = []
            for inst in insts:
                sync = inst.get("sync_info") or {}
                waits = sync.get("on_wait") or []
                cap = _max_waits(inst["opcode"])
                if len(waits) > cap:
                    extra = waits[: len(waits) - cap]
                    keep = waits[len(waits) - cap :]
                    while extra:
                        chunk, extra = extra[:1], extra[1:]
                        out.append(
                            {
                                "debug": inst.get("debug", 0),
                                "engine": inst["engine"],
                                "ins": [],
                                "name": f"{inst['name']}-wsplit{n_new}",
                                "opcode": "Drain",
                                "outs": [],
                                "sync_info": {
                                    "on_update": [],
                                    "on_wait": chunk,
                                },
                            }
                        )
                        n_new += 1
                    sync["on_wait"] = keep
                    inst["sync_info"] = sync
                out.append(inst)
            bb["instructions"] = out
    return json.dumps(bir).encode()


def _patch_json(nc):
    orig = nc.to_json_bytes

    def patched():
        return _split_excess_waits(orig())

    nc.to_json_bytes = patched
    return nc

L = 64
N = L * L  # 4096
B = 8192
NCORES = 8
BPC = B // NCORES  # 1024 rows per core
P = 128
NTILES = BPC // P  # 8

M_SQ = -4.0
LAM = 6.975
C2 = 2.0 + 0.5 * M_SQ  # == 0.0 for the reference constants
SQRT_LAM = math.sqrt(LAM)

TRACE = False
LAST_EXEC_NS = None

_f32 = mybir.dt.float32
_bf16 = mybir.dt.bfloat16


def _neighbours(length):
    idx = np.arange(length * length).reshape(length, length)
    shifts = [
        np.roll(idx, -1, axis=1),
        np.roll(idx, 1, axis=1),
        np.roll(idx, -1, axis=0),
        np.roll(idx, 1, axis=0),
    ]
    return np.stack([s.reshape(-1) for s in shifts], axis=0)


def _is_canonical_lattice(shift: np.ndarray) -> bool:
    if shift.shape != (4, N):
        return False
    exp = np.sort(_neighbours(L), axis=0)
    got = np.sort(shift.astype(np.int64), axis=0)
    return bool(np.array_equal(exp, got))


def _tile_body(nc, pools, x, res_col, ns=None):
    """Emit one 128-row tile's compute. x: [P, N] phi tile in SBUF.
    If ns is None: lattice fast path (shifted-view products).
    Else: generic path with ns = sum_s phi[:, shift[s]] tile."""
    sqp, junkp, accp = pools
    mult = mybir.AluOpType.mult
    Square = mybir.ActivationFunctionType.Square

    kacc = accp.tile([P, 8], _f32)

    a = sqp.tile([P, N], _f32)
    nc.scalar.square(a, x)
    jact = junkp.tile([P, N], _bf16)
    nc.scalar.activation(
        jact, a, Square, scale=SQRT_LAM, accum_out=kacc[:, 0:1]
    )
    assert C2 == 0.0  # mass term vanishes for the reference constants

    jd = junkp.tile([P, N], _bf16)
    if ns is None:
        x3 = x.rearrange("p (r c) -> p r c", c=L)
        j3 = jd.rearrange("p (r c) -> p r c", c=L)
        # S_x interior: phi[r,c]*phi[r,c+1], c in [0,63)
        nc.vector.scalar_tensor_tensor(
            out=j3[:, :, 0:63], in0=x3[:, :, 1:64], scalar=-1.0,
            in1=x3[:, :, 0:63], op0=mult, op1=mult, accum_out=kacc[:, 1:2])
        # S_x wrap: phi[r,63]*phi[r,0]
        nc.vector.scalar_tensor_tensor(
            out=j3[:, :, 63:64], in0=x3[:, :, 0:1], scalar=-1.0,
            in1=x3[:, :, 63:64], op0=mult, op1=mult, accum_out=kacc[:, 2:3])
        # S_y interior: phi[r,c]*phi[r+1,c]
        nc.vector.scalar_tensor_tensor(
            out=jd[:, 0 : N - L], in0=x[:, L:N], scalar=-1.0,
            in1=x[:, 0 : N - L], op0=mult, op1=mult, accum_out=kacc[:, 3:4])
        # S_y wrap: phi[63,c]*phi[0,c]
        nc.vector.scalar_tensor_tensor(
            out=jd[:, N - L : N], in0=x[:, 0:L], scalar=-1.0,
            in1=x[:, N - L : N], op0=mult, op1=mult, accum_out=kacc[:, 4:5])
        ncols = 5
    else:
        nc.vector.scalar_tensor_tensor(
            out=jd[:, :], in0=ns, scalar=-0.5,
            in1=x, op0=mult, op1=mult, accum_out=kacc[:, 1:2])
        ncols = 2

    nc.vector.reduce_sum(
        out=res_col, in_=kacc[:, 0:ncols], axis=mybir.AxisListType.X
    )


def _build(generic: bool):
    nc = bass.Bass()
    phi = nc.dram_tensor("phi", [BPC, N], _f32, kind="ExternalInput")
    if generic:
        nsum = nc.dram_tensor("nsum", [BPC, N], _f32, kind="ExternalInput")
    act = nc.dram_tensor("act", [BPC, 1], _f32, kind="ExternalOutput")

    with tile.TileContext(nc) as tc:
        with (
            tc.tile_pool(name="io", bufs=3) as io,
            tc.tile_pool(name="sq", bufs=2) as sqp,
            tc.tile_pool(name="junk", bufs=2) as junkp,
            tc.tile_pool(name="accs", bufs=3) as accp,
            tc.tile_pool(name="resp", bufs=1) as resp,
        ):
            res = resp.tile([P, NTILES], _f32)
            for t in range(NTILES):
                x = io.tile([P, N], _f32)
                nc.sync.dma_start(out=x, in_=phi[t * P : (t + 1) * P, :])
                ns = None
                if generic:
                    ns = io.tile([P, N], _f32)
                    nc.sync.dma_start(
                        out=ns, in_=nsum[t * P : (t + 1) * P, :]
                    )
                _tile_body(
                    nc, (sqp, junkp, accp), x, res[:, t : t + 1], ns=ns
                )

            out_view = act[:, :].rearrange("(t p) o -> p (t o)", p=P)
            nc.sync.dma_start(out=out_view, in_=res)
    return nc


_cache = {}


def _get(generic: bool):
    if generic not in _cache:
        _cache[generic] = _patch_json(_build(generic))
    return _cache[generic]


def kernel(phi_state, shift):
    global LAST_EXEC_NS
    phi = np.ascontiguousarray(np.asarray(phi_state, dtype=np.float32))
    assert phi.shape == (B, N), phi.shape
    shift_np = np.asarray(shift)

    if _is_canonical_lattice(shift_np):
        nc = _get(False)
        in_maps = [
            {"phi": phi[i * BPC : (i + 1) * BPC]} for i in range(NCORES)
        ]
    else:
        nsum = np.zeros_like(phi)
        for s in range(shift_np.shape[0]):
            nsum += phi[:, shift_np[s].astype(np.int64)]
        nc = _get(True)
        in_maps = [
            {
                "phi": phi[i * BPC : (i + 1) * BPC],
                "nsum": nsum[i * BPC : (i + 1) * BPC],
            }
            for i in range(NCORES)
        ]

    r = run_bass_kernel_spmd(
        nc, in_maps, core_ids=list(range(NCORES)), trace=TRACE
    )
    LAST_EXEC_NS = r.exec_time_ns
    out = np.concatenate([m["act"] for m in r.results], axis=0)
    return out.astype(np.float32)


# revision 12
# speedup vs baseline: 1.0039x; 1.0039x over previous
"""Phi^4 lattice action on Trainium2 (Bass/Tile), 8-core data parallel.

out[b] = sum_i [ (2 + 0.5*M_SQ)*phi^2 + LAM*phi^4 ]
         - 0.5 * sum_{i,s} phi[b,i]*phi[b,shift[s,i]]

For the canonical 64x64 periodic-lattice shift set {+x,-x,+y,-y} the kinetic
term equals -(S_x + S_y) with S_x = sum_i phi[i]*phi[roll_x(i)] (shift-sum
symmetry over the torus), so the gather reduces to two shifted-view products
that are pure access patterns in SBUF - no data movement.

Per 128-row batch tile:
  ACT: a = phi^2 ; junk = Square(sqrt(LAM)*a) with accum_out -> LAM*sum phi^4
  DVE: 4x scalar_tensor_tensor (x-main, x-wrap, y-main, y-wrap), each fused
       multiply(+negate)+reduce into one accumulator column, then one tiny
       reduce over the 5 partial columns.
"""

import json
import math

import numpy as np

import concourse.bass as bass
import concourse.mybir as mybir
import concourse.tile as tile
from concourse.bass_utils import run_bass_kernel_spmd

def _max_waits(opcode: str) -> int:
    # This walrus build accepts at most ONE sync wait per instruction.
    return 1


def _split_excess_waits(bir_bytes: bytes) -> bytes:
    """The container's walrus codegen rejects any instruction carrying more
    than 2 sync waits ("Too many sync wait commands"), but Tile's tail drain
    and WAR-gated DMA loads can carry 3+. Peel excess waits onto injected
    same-engine Drain instructions placed immediately before the offender."""
    bir = json.loads(bir_bytes)
    n_new = 0
    for func in bir.get("functions", []):
        for bb in func.get("blocks", []):
            insts = bb.get("instructions", [])
            out = []
            for inst in insts:
                sync = inst.get("sync_info") or {}
                waits = sync.get("on_wait") or []
                cap = _max_waits(inst["opcode"])
                if len(waits) > cap:
                    extra = waits[: len(waits) - cap]
                    keep = waits[len(waits) - cap :]
                    while extra:
                        chunk, extra = extra[:1], extra[1:]
                        out.append(
                            {
                                "debug": inst.get("debug", 0),
                                "engine": inst["engine"],
                                "ins": [],
                                "name": f"{inst['name']}-wsplit{n_new}",
                                "opcode": "Drain",
                                "outs": [],
                                "sync_info": {
                                    "on_update": [],
                                    "on_wait": chunk,
                                },
                            }
                        )
                        n_new += 1
                    sync["on_wait"] = keep
                    inst["sync_info"] = sync
                out.append(inst)
            bb["instructions"] = out
    return json.dumps(bir).encode()


def _patch_json(nc):
    orig = nc.to_json_bytes

    def patched():
        return _split_excess_waits(orig())

    nc.to_json_bytes = patched
    return nc

L = 64
N = L * L  # 4096
B = 8192
NCORES = 8
BPC = B // NCORES  # 1024 rows per core
P = 128
NTILES = BPC // P  # 8

M_SQ = -4.0
LAM = 6.975
C2 = 2.0 + 0.5 * M_SQ  # == 0.0 for the reference constants
SQRT_LAM = math.sqrt(LAM)

TRACE = False
LAST_EXEC_NS = None

_f32 = mybir.dt.float32
_bf16 = mybir.dt.bfloat16


def _neighbours(length):
    idx = np.arange(length * length).reshape(length, length)
    shifts = [
        np.roll(idx, -1, axis=1),
        np.roll(idx, 1, axis=1),
        np.roll(idx, -1, axis=0),
        np.roll(idx, 1, axis=0),
    ]
    return np.stack([s.reshape(-1) for s in shifts], axis=0)


def _is_canonical_lattice(shift: np.ndarray) -> bool:
    if shift.shape != (4, N):
        return False
    exp = np.sort(_neighbours(L), axis=0)
    got = np.sort(shift.astype(np.int64), axis=0)
    return bool(np.array_equal(exp, got))


def _tile_body(nc, pools, x, kacc, c0, ns=None):
    """Emit one 128-row tile's compute. x: [P, N] phi tile in SBUF.
    Partial sums land in kacc columns c0..c0+6 (combined at the end).
    If ns is None: lattice fast path (shifted-view products), with the
    x/y interior products split into halves gated on the half-loads.
    Else: generic path with ns = sum_s phi[:, shift[s]] tile."""
    sqp, junkp = pools
    mult = mybir.AluOpType.mult
    Square = mybir.ActivationFunctionType.Square

    a = sqp.tile([P, N], _f32)
    nc.scalar.square(a, x)
    jact = junkp.tile([P, N], _bf16)
    nc.scalar.activation(
        jact, a, Square, scale=SQRT_LAM, accum_out=kacc[:, c0 : c0 + 1]
    )
    assert C2 == 0.0  # mass term vanishes for the reference constants

    jd = junkp.tile([P, N], _bf16)

    def stt(out, in0, in1, col):
        nc.vector.scalar_tensor_tensor(
            out=out, in0=in0, scalar=-1.0, in1=in1,
            op0=mult, op1=mult, accum_out=kacc[:, c0 + col : c0 + col + 1])

    if ns is None:
        H = N // 2
        x3 = x.rearrange("p (r c) -> p r c", c=L)
        j3 = jd.rearrange("p (r c) -> p r c", c=L)
        R = L // 2
        # S_x interior, rows [0,32) then [32,64)
        stt(j3[:, 0:R, 0:63], x3[:, 0:R, 1:64], x3[:, 0:R, 0:63], 1)
        stt(j3[:, R:L, 0:63], x3[:, R:L, 1:64], x3[:, R:L, 0:63], 2)
        # S_y interior, pairs (i, i+L): first half then second half
        stt(jd[:, 0 : H - L], x[:, L:H], x[:, 0 : H - L], 3)
        stt(jd[:, H - L : N - L], x[:, H:N], x[:, H - L : N - L], 4)
        # wraps
        stt(j3[:, :, 63:64], x3[:, :, 0:1], x3[:, :, 63:64], 5)
        stt(jd[:, N - L : N], x[:, 0:L], x[:, N - L : N], 6)
    else:
        nc.vector.scalar_tensor_tensor(
            out=jd[:, :], in0=ns, scalar=-0.5,
            in1=x, op0=mult, op1=mult,
            accum_out=kacc[:, c0 + 1 : c0 + 2])


def _build(generic: bool):
    nc = bass.Bass()
    phi = nc.dram_tensor("phi", [BPC, N], _f32, kind="ExternalInput")
    if generic:
        nsum = nc.dram_tensor("nsum", [BPC, N], _f32, kind="ExternalInput")
    act = nc.dram_tensor("act", [BPC, 1], _f32, kind="ExternalOutput")

    CPT = 8  # kacc columns per tile (0: s4, 1-6: kinetic partials)
    H = N // 2
    with tile.TileContext(nc) as tc:
        with (
            tc.tile_pool(name="io", bufs=4) as io,
            tc.tile_pool(name="sq", bufs=2) as sqp,
            tc.tile_pool(name="junk", bufs=2) as junkp,
            tc.tile_pool(name="accs", bufs=1) as accp,
            tc.tile_pool(name="resp", bufs=1) as resp,
        ):
            kacc = accp.tile([P, NTILES * CPT], _f32)
            nc.vector.memset(kacc, 0.0)  # generic path leaves some cols unwritten
            res = resp.tile([P, NTILES], _f32)
            for t in range(NTILES):
                x = io.tile([P, N], _f32)
                # split the load so compute can start on the first half
                nc.sync.dma_start(
                    out=x[:, 0:H], in_=phi[t * P : (t + 1) * P, 0:H]
                )
                nc.sync.dma_start(
                    out=x[:, H:N], in_=phi[t * P : (t + 1) * P, H:N]
                )
                ns = None
                if generic:
                    ns = io.tile([P, N], _f32)
                    nc.sync.dma_start(
                        out=ns, in_=nsum[t * P : (t + 1) * P, :]
                    )
                _tile_body(nc, (sqp, junkp), x, kacc, t * CPT, ns=ns)

            kview = kacc.rearrange("p (t c) -> p t c", c=CPT)
            nc.vector.reduce_sum(
                out=res, in_=kview[:, :, 0:7], axis=mybir.AxisListType.X
            )
            out_view = act[:, :].rearrange("(t p) o -> p (t o)", p=P)
            nc.sync.dma_start(out=out_view, in_=res)
    return nc


_cache = {}


def _get(generic: bool):
    if generic not in _cache:
        _cache[generic] = _patch_json(_build(generic))
    return _cache[generic]


def kernel(phi_state, shift):
    global LAST_EXEC_NS
    phi = np.ascontiguousarray(np.asarray(phi_state, dtype=np.float32))
    assert phi.shape == (B, N), phi.shape
    shift_np = np.asarray(shift)

    if _is_canonical_lattice(shift_np):
        nc = _get(False)
        in_maps = [
            {"phi": phi[i * BPC : (i + 1) * BPC]} for i in range(NCORES)
        ]
    else:
        nsum = np.zeros_like(phi)
        for s in range(shift_np.shape[0]):
            nsum += phi[:, shift_np[s].astype(np.int64)]
        nc = _get(True)
        in_maps = [
            {
                "phi": phi[i * BPC : (i + 1) * BPC],
                "nsum": nsum[i * BPC : (i + 1) * BPC],
            }
            for i in range(NCORES)
        ]

    r = run_bass_kernel_spmd(
        nc, in_maps, core_ids=list(range(NCORES)), trace=TRACE
    )
    LAST_EXEC_NS = r.exec_time_ns
    out = np.concatenate([m["act"] for m in r.results], axis=0)
    return out.astype(np.float32)


# revision 13
# speedup vs baseline: 1.1041x; 1.0998x over previous
"""Phi^4 lattice action on Trainium2 (Bass/Tile), 8-core data parallel.

out[b] = sum_i [ (2 + 0.5*M_SQ)*phi^2 + LAM*phi^4 ]
         - 0.5 * sum_{i,s} phi[b,i]*phi[b,shift[s,i]]

For the canonical 64x64 periodic-lattice shift set {+x,-x,+y,-y} the kinetic
term equals -(S_x + S_y) with S_x = sum_i phi[i]*phi[roll_x(i)] (shift-sum
symmetry over the torus), so the gather reduces to two shifted-view products
that are pure access patterns in SBUF - no data movement.

Per 128-row batch tile:
  ACT: a = phi^2 ; junk = Square(sqrt(LAM)*a) with accum_out -> LAM*sum phi^4
  DVE: 4x scalar_tensor_tensor (x-main, x-wrap, y-main, y-wrap), each fused
       multiply(+negate)+reduce into one accumulator column, then one tiny
       reduce over the 5 partial columns.
"""

import json
import math

import numpy as np

import concourse.bass as bass
import concourse.mybir as mybir
import concourse.tile as tile
from concourse.bass_utils import run_bass_kernel_spmd

def _max_waits(opcode: str) -> int:
    # This walrus build accepts at most ONE sync wait per instruction.
    return 1


def _split_excess_waits(bir_bytes: bytes) -> bytes:
    """The container's walrus codegen rejects any instruction carrying more
    than 2 sync waits ("Too many sync wait commands"), but Tile's tail drain
    and WAR-gated DMA loads can carry 3+. Peel excess waits onto injected
    same-engine Drain instructions placed immediately before the offender."""
    bir = json.loads(bir_bytes)
    n_new = 0
    for func in bir.get("functions", []):
        for bb in func.get("blocks", []):
            insts = bb.get("instructions", [])
            out = []
            for inst in insts:
                sync = inst.get("sync_info") or {}
                waits = sync.get("on_wait") or []
                cap = _max_waits(inst["opcode"])
                if len(waits) > cap:
                    extra = waits[: len(waits) - cap]
                    keep = waits[len(waits) - cap :]
                    while extra:
                        chunk, extra = extra[:1], extra[1:]
                        out.append(
                            {
                                "debug": inst.get("debug", 0),
                                "engine": inst["engine"],
                                "ins": [],
                                "name": f"{inst['name']}-wsplit{n_new}",
                                "opcode": "Drain",
                                "outs": [],
                                "sync_info": {
                                    "on_update": [],
                                    "on_wait": chunk,
                                },
                            }
                        )
                        n_new += 1
                    sync["on_wait"] = keep
                    inst["sync_info"] = sync
                out.append(inst)
            bb["instructions"] = out
    return json.dumps(bir).encode()


def _patch_json(nc):
    orig = nc.to_json_bytes

    def patched():
        return _split_excess_waits(orig())

    nc.to_json_bytes = patched
    return nc

L = 64
N = L * L  # 4096
B = 8192
NCORES = 8
BPC = B // NCORES  # 1024 rows per core
P = 128
NTILES = BPC // P  # 8

M_SQ = -4.0
LAM = 6.975
C2 = 2.0 + 0.5 * M_SQ  # == 0.0 for the reference constants
SQRT_LAM = math.sqrt(LAM)

TRACE = False
LAST_EXEC_NS = None

_f32 = mybir.dt.float32
_bf16 = mybir.dt.bfloat16


def _neighbours(length):
    idx = np.arange(length * length).reshape(length, length)
    shifts = [
        np.roll(idx, -1, axis=1),
        np.roll(idx, 1, axis=1),
        np.roll(idx, -1, axis=0),
        np.roll(idx, 1, axis=0),
    ]
    return np.stack([s.reshape(-1) for s in shifts], axis=0)


def _is_canonical_lattice(shift: np.ndarray) -> bool:
    if shift.shape != (4, N):
        return False
    exp = np.sort(_neighbours(L), axis=0)
    got = np.sort(shift.astype(np.int64), axis=0)
    return bool(np.array_equal(exp, got))


def _tile_body(nc, pools, x, kacc, c0, ns=None, split=False):
    """Emit one 128-row tile's compute. x: [P, N] phi tile in SBUF.
    Partial sums land in kacc columns c0..c0+6 (combined at the end).
    If ns is None: lattice fast path (shifted-view products), with the
    x/y interior products split into halves gated on the half-loads.
    Else: generic path with ns = sum_s phi[:, shift[s]] tile."""
    sqp, junkp = pools
    mult = mybir.AluOpType.mult
    Square = mybir.ActivationFunctionType.Square

    a = sqp.tile([P, N], _f32)
    nc.scalar.square(a, x)
    jact = junkp.tile([P, N], _bf16)
    nc.scalar.activation(
        jact, a, Square, scale=SQRT_LAM, accum_out=kacc[:, c0 : c0 + 1]
    )
    assert C2 == 0.0  # mass term vanishes for the reference constants

    jd = junkp.tile([P, N], _bf16)

    def stt(out, in0, in1, col):
        nc.vector.scalar_tensor_tensor(
            out=out, in0=in0, scalar=-1.0, in1=in1,
            op0=mult, op1=mult, accum_out=kacc[:, c0 + col : c0 + col + 1])

    if ns is None:
        H = N // 2
        x3 = x.rearrange("p (r c) -> p r c", c=L)
        j3 = jd.rearrange("p (r c) -> p r c", c=L)
        R = L // 2
        if split:
            # S_x interior, rows [0,32) then [32,64) - gated on half-loads
            stt(j3[:, 0:R, 0:63], x3[:, 0:R, 1:64], x3[:, 0:R, 0:63], 1)
            stt(j3[:, R:L, 0:63], x3[:, R:L, 1:64], x3[:, R:L, 0:63], 2)
            stt(jd[:, 0 : H - L], x[:, L:H], x[:, 0 : H - L], 3)
            stt(jd[:, H - L : N - L], x[:, H:N], x[:, H - L : N - L], 4)
        else:
            stt(j3[:, :, 0:63], x3[:, :, 1:64], x3[:, :, 0:63], 1)
            stt(jd[:, 0 : N - L], x[:, L:N], x[:, 0 : N - L], 3)
        # wraps
        stt(j3[:, :, 63:64], x3[:, :, 0:1], x3[:, :, 63:64], 5)
        stt(jd[:, N - L : N], x[:, 0:L], x[:, N - L : N], 6)
    else:
        nc.vector.scalar_tensor_tensor(
            out=jd[:, :], in0=ns, scalar=-0.5,
            in1=x, op0=mult, op1=mult,
            accum_out=kacc[:, c0 + 1 : c0 + 2])


def _build(generic: bool):
    nc = bass.Bass()
    phi = nc.dram_tensor("phi", [BPC, N], _f32, kind="ExternalInput")
    if generic:
        nsum = nc.dram_tensor("nsum", [BPC, N], _f32, kind="ExternalInput")
    # [P, NTILES] so the store is contiguous per partition line; the host
    # transposes (act[p, t] = batch row t*P + p).
    act = nc.dram_tensor("act", [P, NTILES], _f32, kind="ExternalOutput")

    CPT = 8  # kacc columns per tile (0: s4, 1-6: kinetic partials)
    H = N // 2
    with tile.TileContext(nc) as tc:
        with (
            tc.tile_pool(name="io", bufs=4) as io,
            tc.tile_pool(name="sq", bufs=2) as sqp,
            tc.tile_pool(name="junk", bufs=2) as junkp,
            tc.tile_pool(name="accs", bufs=1) as accp,
            tc.tile_pool(name="resp", bufs=1) as resp,
        ):
            kacc = accp.tile([P, NTILES * CPT], _f32)
            nc.vector.memset(kacc, 0.0)  # generic path leaves some cols unwritten
            res = resp.tile([P, NTILES], _f32)
            for t in range(NTILES):
                x = io.tile([P, N], _f32)
                # split the load so compute can start on the first half
                nc.sync.dma_start(
                    out=x[:, 0:H], in_=phi[t * P : (t + 1) * P, 0:H]
                )
                nc.sync.dma_start(
                    out=x[:, H:N], in_=phi[t * P : (t + 1) * P, H:N]
                )
                ns = None
                if generic:
                    ns = io.tile([P, N], _f32)
                    nc.sync.dma_start(
                        out=ns, in_=nsum[t * P : (t + 1) * P, :]
                    )
                _tile_body(nc, (sqp, junkp), x, kacc, t * CPT, ns=ns, split=(t == 0))

            kview = kacc.rearrange("p (t c) -> p t c", c=CPT)
            nc.vector.reduce_sum(
                out=res, in_=kview[:, :, 0:7], axis=mybir.AxisListType.X
            )
            nc.sync.dma_start(out=act[:, :], in_=res)
    return nc


_cache = {}


def _get(generic: bool):
    if generic not in _cache:
        _cache[generic] = _patch_json(_build(generic))
    return _cache[generic]


def kernel(phi_state, shift):
    global LAST_EXEC_NS
    phi = np.ascontiguousarray(np.asarray(phi_state, dtype=np.float32))
    assert phi.shape == (B, N), phi.shape
    shift_np = np.asarray(shift)

    if _is_canonical_lattice(shift_np):
        nc = _get(False)
        in_maps = [
            {"phi": phi[i * BPC : (i + 1) * BPC]} for i in range(NCORES)
        ]
    else:
        nsum = np.zeros_like(phi)
        for s in range(shift_np.shape[0]):
            nsum += phi[:, shift_np[s].astype(np.int64)]
        nc = _get(True)
        in_maps = [
            {
                "phi": phi[i * BPC : (i + 1) * BPC],
                "nsum": nsum[i * BPC : (i + 1) * BPC],
            }
            for i in range(NCORES)
        ]

    r = run_bass_kernel_spmd(
        nc, in_maps, core_ids=list(range(NCORES)), trace=TRACE
    )
    LAST_EXEC_NS = r.exec_time_ns
    out = np.concatenate(
        [m["act"].T.reshape(BPC, 1) for m in r.results], axis=0
    )
    return out.astype(np.float32)
